# revision 20
# baseline (speedup 1.0000x reference)
"""Attention-pooling kernel for Trainium2 (8 NeuronCores, data parallel).

Computes, for full inputs query [B, D], keys [B, L, D], W [1, D]:
    inter  = keys * query[:, None, :]
    scores = tanh(einsum('bld,od->blo', inter, W))
    p      = softmax(scores, axis=1)
    out    = sum(p * keys, axis=1)                      # [B, D]

Sharding: batch dim split evenly across 8 cores; W replicated.

Per-core layout: batch rows on partitions, [128, L*D] tiles. DVE does the
two elementwise multiplies and the segmented reductions; ACT does
tanh/exp (+ softmax denominator via accum_out).
"""

import sys

if "/opt/trn_rl_repo" not in sys.path:
    sys.path.insert(0, "/opt/trn_rl_repo")

import numpy as np

import concourse.bacc as bacc
import concourse.bass as bass
import concourse.mybir as mybir
import concourse.tile as tile
from concourse.bass_utils import run_bass_kernel_spmd

B, L, D = 16384, 200, 64
NCORES = 8
BC = B // NCORES  # batch rows per core
PT = 128          # partition tile (batch rows per SBUF tile)
NT = BC // PT     # tiles per core

# variant = (keys_bf16, d_tree, l_tree[, pe2, rk1, hwcast])  or "v2"
FAST_VARIANT = "v2"
OLD_FAST_VARIANT = (True, False, True, True, True)
SAFE_VARIANT = (False, False, False)
DEFAULT_VARIANT = SAFE_VARIANT

_cache = {}
_run_state = {"variant": FAST_VARIANT, "checked": False}
DGA_ON_POOL = True
OF_ON_POOL = True


def _tree_reduce_mid(nc, pool, src_ap, g, n_mid, two, dtype, out_ap, tag):
    """Sum over the MIDDLE axis of a [PT, g, n_mid, two] view via halving
    tensor_tensor adds (innermost pair stays stride-1 -> 2x mode for bf16).
    Final [PT, g*two] f32 result lands in out_ap (natural d order when the
    view is pair-major d)."""
    cur = src_ap
    n = n_mid
    lvl = 0
    while n > 1:
        h, odd = n // 2, n % 2
        if h + odd == 1:
            nc.vector.tensor_add(
                out_ap.rearrange("p (g two) -> p g two", g=g).unsqueeze(2),
                cur[:, :, 0:1, :],
                cur[:, :, 1:2, :],
            )
            return
        t = pool.tile([PT, g * (h + odd) * two], dtype, tag=f"{tag}{lvl % 2}")
        dst = t[:].rearrange("p (g n two) -> p g n two", g=g, two=two)
        nc.vector.tensor_add(
            dst[:, :, 0:h, :], cur[:, :, 0:h, :], cur[:, :, h : 2 * h, :]
        )
        if odd:
            nc.vector.tensor_copy(dst[:, :, h : h + 1, :], cur[:, :, 2 * h : n, :])
        cur = dst
        n = h + odd
        lvl += 1


def _build_v2():
    """Pair-major transpose on ACT + stride-2 rank-1 diag matmuls on PE.

    Per 128-row tile:
      1. SWDGE cast-DMA keys f32->bf16 into kt [P, L*D] (HBM-roofline).
      2. ACT strided f32-pair copy: kt [P, L, 32pair] -> ktp [P, 32pair, L]
         (d-pair-major; PE rhs becomes stride-2 = full streaming rate).
      3. DVE builds 32-wide diag blocks dga32[p, d, j] = v[p, d] * [j==p%32].
      4. PE: per d, four concurrent 32x32 tile_position diag matmuls,
         rhs = ktp[:, d//2, :, d%2], accumulating scores [P, L] in PSUM.
      5. ACT: tanh; exp (accum S); ln S; exp(th - lnS) twice into bf16
         pairs (softmax fully normalized on ACT, no reciprocal).
      6. DVE: wk = ktp * p (pair-major, 2x) ; halving tree over l; out DMA.
    """
    f32 = mybir.dt.float32
    bf16 = mybir.dt.bfloat16
    AF = mybir.ActivationFunctionType

    nc = bacc.Bacc("TRN2", target_bir_lowering=False, debug=False, num_devices=NCORES)
    q_h = nc.declare_dram_parameter("query", [BC, D], f32, isOutput=False)
    k_h = nc.declare_dram_parameter("keys", [BC, L, D], f32, isOutput=False)
    w_h = nc.declare_dram_parameter("W", [PT, D], f32, isOutput=False)
    e_h = nc.declare_dram_parameter("eye", [PT, 32], bf16, isOutput=False)
    o_h = nc.declare_dram_parameter("out", [BC, D], f32, isOutput=True)

    G = D // 2  # 32 d-pairs

    with tile.TileContext(nc) as tc:
        with (
            tc.tile_pool(name="keys", bufs=2) as kp,
            tc.tile_pool(name="ktp", bufs=3) as tpp,
            tc.tile_pool(name="work", bufs=1) as wp,
            tc.tile_pool(name="tree", bufs=1) as tp,
            tc.tile_pool(name="small", bufs=2) as sp,
            tc.tile_pool(name="diag", bufs=2) as dgp,
            tc.tile_pool(name="psum", bufs=2, space="PSUM") as pp,
            tc.tile_pool(name="const", bufs=1) as cp,
        ):
            eye0 = cp.tile([PT, 32], bf16)
            nc.sync.dma_start(eye0[:], e_h[:])
            eye_t = cp.tile([PT, 32], bf16)
            nc.vector.tensor_copy(eye_t[:], eye0[:])
            wb0 = cp.tile([PT, D], f32)
            nc.sync.dma_start(wb0[:], w_h[:])
            wb = cp.tile([PT, D], f32)
            nc.vector.tensor_copy(wb[:], wb0[:])

            # Software-pipelined: stage A(t) = load + diag build + transpose
            # chunk 0; stage B(t-1) = matmuls + softmax, THEN transpose
            # chunks 1..3 of tile t, then pooling of t-1. The chunked
            # transpose lets tanh(t-1)/exp(t-1) slot into the ACT FIFO after
            # only 1/4 of the 11.5us transpose, unblocking the DVE wk
            # multiply (otherwise softmax serializes behind the whole
            # transpose and DVE idles).
            NCHUNK = 4
            GC = G // NCHUNK  # 8 d-pair groups per transpose chunk

            def _transpose_chunk(ktp, kt, c):
                gs = slice(c * GC, (c + 1) * GC)
                nc.scalar.copy(
                    ktp[:]
                    .bitcast(f32)
                    .rearrange("p (g l) -> p g l", l=L)[:, gs, :],
                    kt[:]
                    .bitcast(f32)
                    .rearrange("p (l g) -> p g l", g=G)[:, gs, :],
                )

            stage = {}
            for it in range(NT + 1):
                if it < NT:
                    t = it
                    rows = slice(t * PT, (t + 1) * PT)
                    # two half-DMAs back-to-back: the second's descriptors
                    # drain while the first's ~2us HBM completion receipt is
                    # pending, keeping the SDMA engines fed between tiles
                    kt = kp.tile([PT, L * D], bf16, tag="keys")
                    H = L // 2 * D
                    nc.gpsimd.dma_start(
                        kt[:, 0:H],
                        k_h[rows, 0 : L // 2].rearrange("b l d -> b (l d)"),
                    )
                    nc.gpsimd.dma_start(
                        kt[:, H : 2 * H],
                        k_h[rows, L // 2 : L].rearrange("b l d -> b (l d)"),
                    )
                    qt = sp.tile([PT, D], f32, tag="q")
                    nc.sync.dma_start(qt[:], q_h[rows, :])

                    # v = q * W ; duplicate into adjacent bf16 pairs
                    vt = sp.tile([PT, D], f32, tag="v")
                    nc.vector.tensor_mul(vt[:], qt[:], wb[:])
                    v2p = sp.tile([PT, 2 * D], bf16, tag="v2p")
                    v2v = v2p[:].rearrange("p (d two) -> p d two", two=2)
                    nc.vector.tensor_copy(v2v[:, :, 0], vt[:])
                    nc.vector.tensor_copy(v2v[:, :, 1], vt[:])

                    # dga32[p, d, j] = eye32[p, j] * v[p, d]
                    dga = dgp.tile([PT, D * 32], bf16, tag="dg")
                    dga_eng = nc.gpsimd if DGA_ON_POOL else nc.vector
                    dga_eng.tensor_mul(
                        dga[:].rearrange("p (d j2 two) -> p d j2 two", d=D, two=2),
                        eye_t[:]
                        .rearrange("p (j2 two) -> p j2 two", two=2)
                        .unsqueeze(1)
                        .broadcast_to([PT, D, 16, 2]),
                        v2v.unsqueeze(2).broadcast_to([PT, D, 16, 2]),
                    )

                    # ACT pair-granularity transpose: [P, L, 32] -> [P, 32, L]
                    # chunk 0 now; chunks 1..3 after tile t-1's softmax
                    # (tile 0 has no preceding softmax: emit all 4 here)
                    ktp = tpp.tile([PT, L * D], bf16, tag="ktp")
                    for c in range(NCHUNK if t == 0 else 1):
                        _transpose_chunk(ktp, kt, c)
                    stage[t] = (dga, ktp, kt)

                if it >= 1:
                    t = it - 1
                    rows = slice(t * PT, (t + 1) * PT)
                    dga, ktp, _kt = stage.pop(t)
                    dg3 = dga[:].rearrange("p (d j) -> p d j", d=D)
                    kp4 = ktp[:].rearrange("p (g l two) -> p g l two", g=G, two=2)

                    # scores via 4 concurrent 32x32 diag blocks per d
                    psc = pp.tile([PT, L], f32, tag="sc")
                    for d in range(D):
                        for i in range(4):
                            s = slice(32 * i, 32 * i + 32)
                            nc.tensor.matmul(
                                psc[s, :],
                                dg3[s, d, :],
                                kp4[s, d // 2, :, d % 2],
                                start=(d == 0),
                                stop=(d == D - 1),
                                tile_position=(32 * i, 32 * i),
                            )

                    # softmax: tanh + exp share one ACT table set (no Ln —
                    # a Ln would force two 1.3us table reloads per tile);
                    # normalization via DVE reciprocal + final scale.
                    th = sp.tile([PT, L], f32, tag="th")
                    nc.scalar.activation(th[:], psc[:], AF.Tanh)
                    S = sp.tile([PT, 1], f32, tag="S")
                    ped = sp.tile([PT, 2 * L], bf16, tag="pe")
                    p3 = ped[:].rearrange("p (l two) -> p l two", two=2)
                    nc.scalar.activation(p3[:, :, 0], th[:], AF.Exp, accum_out=S[:])
                    nc.scalar.activation(p3[:, :, 1], th[:], AF.Exp)
                    sinv = sp.tile([PT, 1], f32, tag="sinv")
                    nc.vector.reciprocal(sinv[:], S[:])

                    # rest of tile t+1's transpose, behind the softmax in
                    # the ACT FIFO (tile 0 already emitted all its chunks)
                    if it < NT and it >= 1:
                        _dga_n, ktp_n, kt_n = stage[it]
                        for c in range(1, NCHUNK):
                            _transpose_chunk(ktp_n, kt_n, c)

                    # wk[g, l, k] = ktp[g, l, k] * p[l]  (pair-major, 2x)
                    wk = wp.tile([PT, L * D], bf16, tag="work")
                    wk4 = wk[:].rearrange("p (g l two) -> p g l two", g=G, two=2)
                    nc.vector.tensor_mul(
                        wk4,
                        kp4,
                        p3.unsqueeze(1).broadcast_to([PT, G, L, 2]),
                    )

                    # out[b, d] = sum_l wk (tree over middle axis; d-natural)
                    ou = sp.tile([PT, D], f32, tag="ou")
                    _tree_reduce_mid(nc, tp, wk4, G, L, 2, bf16, ou[:], "mtree")
                    of = sp.tile([PT, D], f32, tag="of")
                    of_eng = nc.gpsimd if OF_ON_POOL else nc.vector
                    of_eng.tensor_scalar_mul(of[:], ou[:], sinv[:])
                    nc.sync.dma_start(o_h[rows, :], of[:])

    nc.compile()
    return nc


def _tree_reduce_outer(nc, pool, src_ap, n_outer, inner, dtype, out_ap, tag):
    """Sum over the OUTER axis of a [PT, n_outer, inner] view via halving
    tensor_tensor adds (inner dim stays contiguous, 2x-mode eligible for
    bf16). Final [PT, inner] f32 result lands in out_ap."""
    cur = src_ap
    n = n_outer
    lvl = 0
    while n > 1:
        h, odd = n // 2, n % 2
        if h + odd == 1:
            nc.vector.tensor_add(
                out_ap.unsqueeze(1), cur[:, 0:1, :], cur[:, 1:2, :]
            )
            return
        # ping-pong tags: level k+1 reads level k, so they must coexist
        t = pool.tile([PT, (h + odd) * inner], dtype, tag=f"{tag}{lvl % 2}")
        dst = t[:].rearrange("p (n i) -> p n i", n=h + odd)
        nc.vector.tensor_add(dst[:, 0:h, :], cur[:, 0:h, :], cur[:, h : 2 * h, :])
        if odd:
            nc.vector.tensor_copy(dst[:, h : h + 1, :], cur[:, 2 * h : n, :])
        cur = dst
        n = h + odd
        lvl += 1


def _tree_reduce_inner(nc, pool, src_ap, outer, n_inner, dtype, out_ap, tag):
    """Sum over the INNER axis of a [PT, outer, n_inner] view via halving
    tensor_tensor adds on contiguous inner slices. n_inner must be a power
    of two. Final [PT, outer] f32 result lands in out_ap."""
    cur = src_ap
    n = n_inner
    lvl = 0
    while n > 1:
        h = n // 2
        if h == 1:
            nc.vector.tensor_add(
                out_ap.unsqueeze(2), cur[:, :, 0:1], cur[:, :, 1:2]
            )
            return
        t = pool.tile([PT, outer * h], dtype, tag=f"{tag}{lvl % 2}")
        dst = t[:].rearrange("p (o i) -> p o i", o=outer)
        nc.vector.tensor_add(dst, cur[:, :, 0:h], cur[:, :, h:n])
        cur = dst
        n = h
        lvl += 1


def _build_bass(variant):
    keys_bf16, d_tree, l_tree = variant[:3]
    pe2 = variant[3] if len(variant) > 3 else False
    rk1 = variant[4] if len(variant) > 4 else False
    hwcast = variant[5] if len(variant) > 5 else False  # f32 HWDGE load + DVE convert
    rk4 = variant[6] if len(variant) > 6 else False  # 32x32 block-diag tile_position
    assert not rk1 or keys_bf16, "rank-1 scoring requires bf16 keys"
    f32 = mybir.dt.float32
    bf16 = mybir.dt.bfloat16
    kdt = bf16 if keys_bf16 else f32
    mdt = bf16 if keys_bf16 else f32  # multiply output dtype
    AF = mybir.ActivationFunctionType
    X = mybir.AxisListType.X

    nc = bacc.Bacc("TRN2", target_bir_lowering=False, debug=False, num_devices=NCORES)
    q_h = nc.declare_dram_parameter("query", [BC, D], f32, isOutput=False)
    k_h = nc.declare_dram_parameter("keys", [BC, L, D], f32, isOutput=False)
    w_h = nc.declare_dram_parameter("W", [PT, D], f32, isOutput=False)
    if rk1:
        e_h = nc.declare_dram_parameter(
            "eye", [PT, 32 if rk4 else PT], bf16, isOutput=False
        )
    o_h = nc.declare_dram_parameter("out", [BC, D], f32, isOutput=True)

    with tile.TileContext(nc) as tc:
        with (
            tc.tile_pool(name="keys", bufs=2) as kp,
            tc.tile_pool(name="work", bufs=2) as wp,
            tc.tile_pool(name="tree", bufs=1) as tp,
            tc.tile_pool(name="small", bufs=2) as sp,
            tc.tile_pool(name="diag", bufs=3) as dgp,
            tc.tile_pool(name="psum", bufs=2, space="PSUM") as pp,
            tc.tile_pool(name="const", bufs=1) as cp,
        ):
            if rk1:
                ew = 32 if rk4 else PT
                eye0 = cp.tile([PT, ew], bf16)
                nc.sync.dma_start(eye0[:], e_h[:])
                eye_t = cp.tile([PT, ew], bf16)
                nc.vector.tensor_copy(eye_t[:], eye0[:])
            # W pre-broadcast to all 128 partitions on the host.
            wb0 = cp.tile([PT, D], f32)
            nc.sync.dma_start(wb0[:], w_h[:])
            # Route through a DVE copy so downstream DVE ops depend on it via
            # program order rather than an extra DMA semaphore wait.
            wb = cp.tile([PT, D], f32)
            nc.vector.tensor_copy(wb[:], wb0[:])

            for t in range(NT):
                rows = slice(t * PT, (t + 1) * PT)

                kt = kp.tile(
                    [PT, L * D], kdt, tag="keys",
                    bufs=1 if hwcast else (3 if keys_bf16 else 2),
                )
                if keys_bf16 and hwcast:
                    ktf = kp.tile([PT, L * D], f32, tag="keysf")
                    nc.sync.dma_start(
                        ktf[:], k_h[rows].rearrange("b l d -> b (l d)")
                    )
                    nc.vector.tensor_copy(kt[:], ktf[:])
                elif keys_bf16:
                    # SWDGE cast-DMA: f32 HBM -> bf16 SBUF
                    nc.gpsimd.dma_start(
                        kt[:], k_h[rows].rearrange("b l d -> b (l d)")
                    )
                else:
                    nc.sync.dma_start(
                        kt[:], k_h[rows].rearrange("b l d -> b (l d)")
                    )
                qt = sp.tile([PT, D], f32, tag="q")
                nc.sync.dma_start(qt[:], q_h[rows, :])

                k3 = kt[:].rearrange("p (l d) -> p l d", l=L)

                if rk1:
                    # v = q * W kept f32, then duplicated into adjacent bf16
                    # pairs (v2p[2d], v2p[2d+1]) = v[d] for the paired
                    # broadcast below.
                    vt = sp.tile([PT, D], f32, tag="v")
                    nc.vector.tensor_mul(vt[:], qt[:], wb[:])
                    v2p = sp.tile([PT, 2 * D], bf16, tag="v2p")
                    v2v = v2p[:].rearrange("p (d two) -> p d two", two=2)
                    nc.vector.tensor_copy(v2v[:, :, 0], vt[:])
                    nc.vector.tensor_copy(v2v[:, :, 1], vt[:])
                    # Build all 64 diag(v[:, d]) blocks in one 2x-mode TT:
                    # dg_all[p, d, j] = eye[p, j] * v[p, d]
                    ew = 32 if rk4 else PT
                    dga = dgp.tile([PT, D * ew], bf16, tag="dg", bufs=1 if hwcast else 3)
                    nc.vector.tensor_mul(
                        dga[:].rearrange(
                            "p (d j2 two) -> p d j2 two", d=D, two=2
                        ),
                        eye_t[:]
                        .rearrange("p (j2 two) -> p j2 two", two=2)
                        .unsqueeze(1)
                        .broadcast_to([PT, D, ew // 2, 2]),
                        v2v.unsqueeze(2).broadcast_to([PT, D, ew // 2, 2]),
                    )
                    # scores[b, l] = sum_d v[b, d] * keys[b, l, d] as
                    # accumulating rank-1 diag matmuls on the TensorEngine:
                    # lhsT = diag(v[:, d]), rhs = keys[:, :, d]
                    psc = pp.tile([PT, L], f32, tag="sc")
                    dg3 = dga[:].rearrange("p (d j) -> p d j", d=D)
                    for d in range(D):
                        if rk4:
                            # four concurrent 32x32 diag-block matmuls
                            for i in range(4):
                                s = slice(32 * i, 32 * i + 32)
                                nc.tensor.matmul(
                                    psc[s, :],
                                    dg3[s, d, :],
                                    k3[s, :, d],
                                    start=(d == 0),
                                    stop=(d == D - 1),
                                    tile_position=(32 * i, 32 * i),
                                )
                        else:
                            nc.tensor.matmul(
                                psc[:],
                                dg3[:, d, :],
                                k3[:, :, d],
                                start=(d == 0),
                                stop=(d == D - 1),
                            )
                    scores = psc
                else:
                    # v = q * W  (per-partition [128, 64])
                    vt = sp.tile([PT, D], mdt, tag="v")
                    nc.vector.tensor_mul(vt[:], qt[:], wb[:])

                    # inter = keys * v (v broadcast along l)
                    inter = wp.tile([PT, L * D], mdt, tag="work")
                    i3 = inter[:].rearrange("p (l d) -> p l d", l=L)
                    nc.vector.tensor_mul(
                        i3, k3, vt[:].unsqueeze(1).broadcast_to([PT, L, D])
                    )

                    # scores[b, l] = sum_d inter
                    scores = sp.tile([PT, L], f32, tag="sc")
                    if d_tree:
                        _tree_reduce_inner(nc, tp, i3, L, D, mdt, scores[:], "dtree")
                    else:
                        nc.vector.reduce_sum(scores[:], i3, axis=X)

                # tanh then exp (same ACT table set); accumulate softmax denom
                th = sp.tile([PT, L], f32, tag="th")
                nc.scalar.activation(th[:], scores[:], AF.Tanh)
                S = sp.tile([PT, 1], f32, tag="S")
                wk = wp.tile([PT, L * D], mdt, tag="work")
                w3 = wk[:].rearrange("p (l d) -> p l d", l=L)
                if pe2:
                    # exp weights duplicated into adjacent pairs so the
                    # broadcast-along-d AP has innermost step 1 (4B-aligned
                    # bf16 pair) -> DVE 2x_1P packed mode for the multiply.
                    ped = sp.tile([PT, 2 * L], mdt, tag="pe")
                    p3 = ped[:].rearrange("p (l two) -> p l two", two=2)
                    nc.scalar.activation(p3[:, :, 0], th[:], AF.Exp, accum_out=S[:])
                    nc.scalar.activation(p3[:, :, 1], th[:], AF.Exp)
                    sinv = sp.tile([PT, 1], f32, tag="sinv")
                    nc.vector.reciprocal(sinv[:], S[:])
                    nc.vector.tensor_mul(
                        wk[:].rearrange("p (l d2 two) -> p l d2 two", l=L, two=2),
                        kt[:].rearrange("p (l d2 two) -> p l d2 two", l=L, two=2),
                        p3.unsqueeze(2).broadcast_to([PT, L, D // 2, 2]),
                    )
                else:
                    pe = sp.tile([PT, L], mdt, tag="pe")
                    nc.scalar.activation(pe[:], th[:], AF.Exp, accum_out=S[:])
                    sinv = sp.tile([PT, 1], f32, tag="sinv")
                    nc.vector.reciprocal(sinv[:], S[:])
                    # wk = keys * exp(scores) (broadcast along d)
                    nc.vector.tensor_mul(
                        w3, k3, pe[:].unsqueeze(2).broadcast_to([PT, L, D])
                    )

                # out_unnorm[b, d] = sum_l wk
                ou = sp.tile([PT, D], f32, tag="ou")
                if l_tree:
                    _tree_reduce_outer(nc, tp, w3, L, D, mdt, ou[:], "ltree")
                else:
                    nc.vector.reduce_sum(
                        ou[:],
                        wk[:].rearrange("p (l d) -> p d l", l=L),
                        axis=X,
                    )
                # normalize by softmax denominator
                of = sp.tile([PT, D], f32, tag="of")
                nc.vector.tensor_scalar_mul(of[:], ou[:], sinv[:])
                nc.sync.dma_start(o_h[rows, :], of[:])

    nc.compile()
    return nc


def _get_nc(variant=DEFAULT_VARIANT):
    key = "v2" if variant == "v2" else tuple(variant)
    if key not in _cache:
        _cache[key] = _build_v2() if key == "v2" else _build_bass(key)
    return _cache[key]


def run_sharded(query, keys, W, trace=False, variant=DEFAULT_VARIANT):
    """Run the SPMD kernel; returns (out [B, D], BassKernelResults)."""
    query = np.ascontiguousarray(query, dtype=np.float32)
    keys = np.ascontiguousarray(keys, dtype=np.float32)
    W = np.ascontiguousarray(W, dtype=np.float32)
    nc = _get_nc(variant)
    w_b = np.ascontiguousarray(np.broadcast_to(W.reshape(1, D), (PT, D)))
    extra = {}
    if variant == "v2":
        import ml_dtypes

        e = np.zeros((PT, 32), dtype=ml_dtypes.bfloat16)
        e[np.arange(PT), np.arange(PT) % 32] = 1
        extra["eye"] = e
    elif len(variant) > 4 and variant[4]:
        import ml_dtypes

        if len(variant) > 6 and variant[6]:
            e = np.zeros((PT, 32), dtype=ml_dtypes.bfloat16)
            e[np.arange(PT), np.arange(PT) % 32] = 1
            extra["eye"] = e
        else:
            extra["eye"] = np.eye(PT, dtype=ml_dtypes.bfloat16)
    in_maps = [
        {
            "query": query[i * BC : (i + 1) * BC],
            "keys": keys[i * BC : (i + 1) * BC],
            "W": w_b,
            **extra,
        }
        for i in range(NCORES)
    ]
    res = run_bass_kernel_spmd(nc, in_maps, core_ids=list(range(NCORES)), trace=trace)
    out = np.concatenate([res.results[i]["out"] for i in range(NCORES)], axis=0)
    return out, res


def _spot_check(out, query, keys, W, n=512):
    """Scaled absmax error of a row subset vs a float64 numpy oracle."""
    idx = np.random.default_rng(0).choice(B, n, replace=False)
    q = query[idx].astype(np.float64)
    k = keys[idx].astype(np.float64)
    w = W.reshape(-1).astype(np.float64)
    sc = np.tanh(((k * q[:, None, :]) * w).sum(-1))
    p = np.exp(sc)
    p /= p.sum(1, keepdims=True)
    ref = (p[:, :, None] * k).sum(1)
    return np.abs(out[idx] - ref).max() / max(np.abs(ref).max(), 1e-6)


def kernel(query, keys, W):
    var = _run_state["variant"]
    try:
        out, _ = run_sharded(query, keys, W, trace=False, variant=var)
        if var != SAFE_VARIANT and not _run_state["checked"]:
            _run_state["checked"] = True
            err = _spot_check(out, query, keys, W)
            if not (err <= 2e-2):  # NaN-safe: NaN must also fail
                raise RuntimeError(f"fast-variant accuracy check failed: {err}")
    except Exception:
        if var == SAFE_VARIANT:
            raise
        _run_state["variant"] = SAFE_VARIANT
        out, _ = run_sharded(query, keys, W, trace=False, variant=SAFE_VARIANT)
    return out

